# revision 70
# baseline (speedup 1.0000x reference)
"""CQAttention (BiDAF context-query attention) Trainium2 kernel, v5.

Shapes: C (32,128,1024), Q (32,128,512), W (32768,1,384) -> out (32,512,1024).
Data-parallel across 8 NeuronCores: 4 batches per core, no collectives.

Strategy:
  - All PE matmuls bf16; zero PE transposes (host supplies W^T/Q^T/C^T
    layouts — pure layout prep).
  - F = exp(S^T) == E^T exactly (the per-context bias r cancels in the row
    normalization): S^T matmuls + second exp collapse into two DMA xbar
    transposes of E halves; A/B scale is then just 1/rowsum.
  - Software-pipelined emission so strict-FIFO engine queues never hold
    next-batch early work behind this-batch late work:
      upfront:  all input DMAs; UT (gpsimd); rbias (DVE) for all batches
      stage1(b): S matmuls + exp->E (+rowsum accum) + 1/rowsum bounce
      stage2(b): xbar E->F, G matmuls + Gn
      stage3(b): A/B matmuls, scaling (DVE), C*A/C*B (gpsimd), output DMA
    emitted as s1(0) s2(0) s1(1) s2(1) s3(0) s1(2) s2(2) s3(1) ...
"""

import ml_dtypes
import numpy as np

import concourse.bass as bass
import concourse.bacc as bacc
import concourse.mybir as mybir
from concourse import tile
from concourse.bass_utils import run_bass_kernel_spmd

B, D, CL, QL = 32, 128, 1024, 512
NCORES = 8
BPC = B // NCORES          # batches per core
NK = CL // D               # 8 c-chunks of 128
NJ = QL // D               # 4 q-chunks of 128

# packed bf16 input column offsets
O_QB = 0
O_QT = O_QB + QL
O_WQT = O_QT + NJ * D
O_WQCT = O_WQT + CL
O_WC = O_WQCT + CL
O_CTO = O_WC + NK * D
PKW = O_CTO + NK * (D + 1)   # 5128

F32 = mybir.dt.float32
BF16 = mybir.dt.bfloat16
EXP = mybir.ActivationFunctionType.Exp
BF = ml_dtypes.bfloat16

_NC = None
RUN_KWARGS = {}        # test harness can set e.g. {"trace": True}
LAST_RESULT = None     # last BassKernelResults (for exec_time_ns / trace)


class _Batch:
    """Per-batch tiles and views."""

    def __init__(self, nc, ipool, pool, dram, b, C_d, PK_d, OUT_d):
        self.OUT_d = OUT_d[b]
        self.OB = ipool.tile([D, 4 * CL], F32, tag=f"OB{b}")
        self.pk = ipool.tile([D, PKW], BF16, tag=f"pk{b}")
        nc.sync.dma_start(self.OB[:, 0:CL], C_d[b])
        nc.sync.dma_start(self.pk[:], PK_d[b])
        self.C = self.OB[:, 0:CL]
        self.Qb = self.pk[:, O_QB:O_QB + QL]
        self.Qt = self.pk[:, O_QT:O_QT + NJ * D]
        self.wqT = self.pk[:, O_WQT:O_WQT + CL]
        self.wqcT = self.pk[:, O_WQCT:O_WQCT + CL]
        self.wc = self.pk[:, O_WC:O_WC + NK * D]
        cto = self.pk[:, O_CTO:O_CTO + NK * (D + 1)]
        self.cto_v = cto.rearrange("p (k e) -> p k e", k=NK)
        self.UTt = ipool.tile([D, CL], BF16, tag=f"UTt{b}")
        self.UT = ipool.tile([D, CL], BF16, tag=f"UT{b}")
        self.rmul = ipool.tile([D, NK * D], BF16, tag=f"rmul{b}")
        self.rbias = ipool.tile([D, NK], F32, tag=f"rbias{b}")
        # pooled (bufs=2) per-batch working tiles
        self.E = pool.tile([D, NK * QL], BF16, tag="E")
        self.En = pool.tile([D, NK * QL], BF16, tag="En")
        self.F = pool.tile([D, NJ * CL], BF16, tag="F", bufs=3)
        self.Fr = self.F.rearrange("p (k j c) -> p k j c", k=NK, j=NJ)
        self.Fh = self.F.rearrange("p (h m c) -> p h m c", h=2, m=NK * NJ // 2)
        self.rowsum = pool.tile([D, NK], F32, tag="rowsum")
        self.rsi = pool.tile([D, NK], F32, tag="rsi")
        self.Gn = pool.tile([D, NJ * D], BF16, tag="Gn")
        self.crec = pool.tile([D, NJ], F32, tag="crec")


def _upfront(nc, t):
    # UT = wq^T + wqc^T * C  (GPSIMD, bf16 out) — batch-independent
    nc.gpsimd.tensor_mul(t.UTt[:], t.wqcT[:], t.C)
    nc.gpsimd.tensor_add(t.UT[:], t.UTt[:], t.wqT[:])
    # rbias[c] = sum_d wc * C^T  (DVE, c-part chunks)
    nc.vector.tensor_mul(t.rmul[:], t.wc[:], t.cto_v[:, :, 0:D])
    nc.vector.tensor_reduce(t.rbias[:],
                            t.rmul.rearrange("p (k e) -> p k e", k=NK),
                            axis=mybir.AxisListType.X, op=mybir.AluOpType.add)


def _stage1(nc, t, psS):
    # S chunks -> E = exp(S + rbias), rowsum accum (bf16)
    for k in range(NK):
        ps = psS.tile([D, QL], F32, tag="ps")
        nc.tensor.matmul(ps[:], t.UT[:, k * D:(k + 1) * D], t.Qb[:],
                         start=True, stop=True)
        nc.scalar.activation(t.E[:, k * QL:(k + 1) * QL], ps[:], EXP,
                             bias=t.rbias[:, k:k + 1],
                             accum_out=t.rowsum[:, k:k + 1])
        if k == NK // 2 - 1 or k == NK - 1:
            # normalize this half: En = E * (1/rowsum) per c-partition.
            # En == S1 exactly, so En^T feeds A/B with no output scaling
            # and no rrB broadcast (kills two sync-ring DMAs per batch).
            hh = k // (NK // 2)
            ksl = slice(hh * NK // 2, (hh + 1) * NK // 2)
            nc.vector.reciprocal(t.rsi[:, ksl], t.rowsum[:, ksl])
            for kk in range(hh * NK // 2, (hh + 1) * NK // 2):
                nc.vector.tensor_scalar_mul(
                    t.En[:, kk * QL:(kk + 1) * QL],
                    t.E[:, kk * QL:(kk + 1) * QL], t.rsi[:, kk:kk + 1])


def _stage2x(nc, t):
    # F = En^T (== S1^T) via DMA xbar transpose per half (sync ring)
    for h in range(2):
        nc.sync.dma_start_transpose(
            t.Fh[:, h, :, :], t.En[:, h * 4 * QL:(h + 1) * 4 * QL])


def _stage2g(nc, t, psG):
    # G = E^T @ [C^T|1] per q-chunk; Gn = G/colsum (bf16)
    for j in range(NJ):
        psg = psG.tile([D, D + 1], F32, tag="psg")
        for k in range(NK):
            nc.tensor.matmul(psg[:],
                             t.E[:, k * QL + j * D: k * QL + (j + 1) * D],
                             t.cto_v[:, k, :],
                             start=(k == 0), stop=(k == NK - 1))
        nc.vector.reciprocal(t.crec[:, j:j + 1], psg[:, D:D + 1])
        nc.vector.tensor_scalar_mul(t.Gn[:, j * D:(j + 1) * D], psg[:, 0:D],
                                    t.crec[:, j:j + 1])


def _stage3(nc, t, psAB):
    Asb = t.OB[:, CL:2 * CL]
    CA = t.OB[:, 2 * CL:3 * CL]
    CB = t.OB[:, 3 * CL:4 * CL]
    for h in range(2):
        hs = slice(h * QL, (h + 1) * QL)
        psa = psAB.tile([D, QL], F32, tag="ps")
        for j in range(NJ):
            nc.tensor.matmul(psa[:], t.Qt[:, j * D:(j + 1) * D],
                             t.Fr[:, 4 * h:4 * h + 4, j, :],
                             start=(j == 0), stop=(j == NJ - 1))
        nc.vector.tensor_copy(Asb[:, hs], psa[:])
        nc.vector.tensor_mul(CA[:, hs], psa[:], t.C[:, hs])
        psb = psAB.tile([D, QL], F32, tag="ps")
        for j in range(NJ):
            nc.tensor.matmul(psb[:], t.Gn[:, j * D:(j + 1) * D],
                             t.Fr[:, 4 * h:4 * h + 4, j, :],
                             start=(j == 0), stop=(j == NJ - 1))
        nc.vector.tensor_mul(CB[:, hs], psb[:], t.C[:, hs])
    # (output DMA emitted separately, deferred — see _out)


def _out(nc, t):
    # rows 128:512 (A^T, C*A, C*B); row block 0:128 (C) was written upfront.
    # Emitted two batches late on the sync ring: its deps are long done by
    # then, so it never head-of-line blocks the xbar transposes behind it.
    nc.sync.dma_start(t.OUT_d[D:4 * D].rearrange("(r p) c -> p r c", p=D),
                      t.OB[:, CL:].rearrange("p (r c) -> p r c", r=3))


def _build():
    nc = bacc.Bacc("TRN2", debug=False, num_devices=NCORES)

    C_d = nc.dram_tensor("C", [BPC, D, CL], F32, kind="ExternalInput").ap()
    PK_d = nc.dram_tensor("PK", [BPC, D, PKW], BF16, kind="ExternalInput").ap()
    OUT_d = nc.dram_tensor("OUT", [BPC, 4 * D, CL], F32, kind="ExternalOutput").ap()

    with tile.TileContext(nc) as tc:
        with (
            tc.tile_pool(name="ins", bufs=1) as ipool,
            tc.tile_pool(name="work", bufs=2) as pool,
            tc.tile_pool(name="psS", bufs=3, space="PSUM") as psS,
            tc.tile_pool(name="psG", bufs=2, space="PSUM") as psG,
            tc.tile_pool(name="psAB", bufs=3, space="PSUM") as psAB,
            tc.tile_pool(name="dram", bufs=2, space="DRAM") as dram,
        ):
            ts = [_Batch(nc, ipool, pool, dram, b, C_d, PK_d, OUT_d)
                  for b in range(BPC)]
            for t in ts:
                # C passthrough output can go as soon as C is loaded
                nc.sync.dma_start(t.OUT_d[0:D, :], t.C)
            # HAM warm-up: keep the PE busy through the ~19us load phase so
            # the clock gate is at 8/8 (2.4 GHz) when real matmuls start
            dmyL = ipool.tile([D, D], BF16, tag="dmyL")
            dmyR = ipool.tile([D, QL], BF16, tag="dmyR")
            nc.vector.memset(dmyL[:], 0.0)
            nc.vector.memset(dmyR[:], 0.0)
            for _ in range(36):
                psd = psS.tile([D, QL], F32, tag="ps")
                nc.tensor.matmul(psd[:], dmyL[:], dmyR[:],
                                 start=True, stop=True)
            for t in ts:
                _upfront(nc, t)
            # Pipeline depth 3: when each instruction reaches its engine
            # queue head, its deps are already satisfied — PE never stalls.
            # PE order: S_b, G_{b-1}, AB_{b-2}, S_{b+1}, ...
            for b in range(BPC):
                _stage1(nc, ts[b], psS)
                _stage2x(nc, ts[b])
                _stage2g(nc, ts[b], psG)
                if b >= 1:
                    _stage3(nc, ts[b - 1], psAB)
                    _out(nc, ts[b - 1])
            _stage3(nc, ts[BPC - 1], psAB)
            _out(nc, ts[BPC - 1])
    nc.compile()
    return nc


def _get_nc():
    global _NC
    if _NC is None:
        _NC = _build()
    return _NC


def _prep_core(Ci, Qi, Wi):
    """Host-side layout prep for one core's shard (pure transposes/casts)."""
    bpc = Ci.shape[0]
    pk = np.empty((bpc, D, PKW), dtype=BF)
    pk[:, :, O_QB:O_QB + QL] = Qi.astype(BF)
    qt = Qi.transpose(0, 2, 1).reshape(bpc, NJ, D, D).transpose(0, 2, 1, 3)
    pk[:, :, O_QT:O_QT + NJ * D] = qt.reshape(bpc, D, NJ * D).astype(BF)
    pk[:, :, O_WQT:O_WQT + CL] = Wi[:, :, 0:D].transpose(0, 2, 1).astype(BF)
    pk[:, :, O_WQCT:O_WQCT + CL] = (
        Wi[:, :, 2 * D:3 * D].transpose(0, 2, 1).astype(BF))
    pk[:, :, O_WC:O_WC + NK * D] = (
        Wi[:, :, D:2 * D].reshape(bpc, NK, D, D).transpose(0, 2, 1, 3)
        .reshape(bpc, D, NK * D).astype(BF))
    ct = Ci.transpose(0, 2, 1).reshape(bpc, NK, D, D).transpose(0, 2, 1, 3)
    cto = np.concatenate(
        [ct, np.ones((bpc, D, NK, 1), dtype=np.float32)], axis=3)
    pk[:, :, O_CTO:O_CTO + NK * (D + 1)] = (
        cto.reshape(bpc, D, NK * (D + 1)).astype(BF))
    return {"C": np.ascontiguousarray(Ci), "PK": pk}


def kernel(C, Q, W):
    C = np.ascontiguousarray(np.asarray(C, dtype=np.float32))
    Q = np.ascontiguousarray(np.asarray(Q, dtype=np.float32))
    W = np.ascontiguousarray(np.asarray(W, dtype=np.float32)).reshape(B, CL, 3 * D)
    in_maps = [
        _prep_core(C[i * BPC:(i + 1) * BPC],
                   Q[i * BPC:(i + 1) * BPC],
                   W[i * BPC:(i + 1) * BPC])
        for i in range(NCORES)
    ]
    nc = _get_nc()
    res = run_bass_kernel_spmd(nc, in_maps, core_ids=list(range(NCORES)), **RUN_KWARGS)
    global LAST_RESULT
    LAST_RESULT = res
    out = np.concatenate([res.results[i]["OUT"] for i in range(NCORES)], axis=0)
    return out


# revision 71
# speedup vs baseline: 1.0461x; 1.0461x over previous
"""CQAttention (BiDAF context-query attention) Trainium2 kernel, v5.

Shapes: C (32,128,1024), Q (32,128,512), W (32768,1,384) -> out (32,512,1024).
Data-parallel across 8 NeuronCores: 4 batches per core, no collectives.

Strategy:
  - All PE matmuls bf16; zero PE transposes (host supplies W^T/Q^T/C^T
    layouts — pure layout prep).
  - F = exp(S^T) == E^T exactly (the per-context bias r cancels in the row
    normalization): S^T matmuls + second exp collapse into two DMA xbar
    transposes of E halves; A/B scale is then just 1/rowsum.
  - Software-pipelined emission so strict-FIFO engine queues never hold
    next-batch early work behind this-batch late work:
      upfront:  all input DMAs; UT (gpsimd); rbias (DVE) for all batches
      stage1(b): S matmuls + exp->E (+rowsum accum) + 1/rowsum bounce
      stage2(b): xbar E->F, G matmuls + Gn
      stage3(b): A/B matmuls, scaling (DVE), C*A/C*B (gpsimd), output DMA
    emitted as s1(0) s2(0) s1(1) s2(1) s3(0) s1(2) s2(2) s3(1) ...
"""

import ml_dtypes
import numpy as np

import concourse.bass as bass
import concourse.bacc as bacc
import concourse.mybir as mybir
from concourse import tile
from concourse.bass_utils import run_bass_kernel_spmd

B, D, CL, QL = 32, 128, 1024, 512
NCORES = 8
BPC = B // NCORES          # batches per core
NK = CL // D               # 8 c-chunks of 128
NJ = QL // D               # 4 q-chunks of 128

# packed bf16 input column offsets
O_QB = 0
O_QT = O_QB + QL
O_WQT = O_QT + NJ * D
O_WQCT = O_WQT + CL
O_WC = O_WQCT + CL
O_CTO = O_WC + NK * D
PKW = O_CTO + NK * (D + 1)   # 5128

F32 = mybir.dt.float32
BF16 = mybir.dt.bfloat16
EXP = mybir.ActivationFunctionType.Exp
BF = ml_dtypes.bfloat16

_NC = None
RUN_KWARGS = {}        # test harness can set e.g. {"trace": True}
LAST_RESULT = None     # last BassKernelResults (for exec_time_ns / trace)


class _Batch:
    """Per-batch tiles and views."""

    def __init__(self, nc, ipool, pool, dram, b, C_d, PK_d, OUT_d):
        self.OUT_d = OUT_d[b]
        self.OB = ipool.tile([D, 4 * CL], F32, tag=f"OB{b}")
        self.pk = ipool.tile([D, PKW], BF16, tag=f"pk{b}")
        nc.sync.dma_start(self.OB[:, 0:CL], C_d[b])
        nc.sync.dma_start(self.pk[:], PK_d[b])
        self.C = self.OB[:, 0:CL]
        self.Qb = self.pk[:, O_QB:O_QB + QL]
        self.Qt = self.pk[:, O_QT:O_QT + NJ * D]
        self.wqT = self.pk[:, O_WQT:O_WQT + CL]
        self.wqcT = self.pk[:, O_WQCT:O_WQCT + CL]
        self.wc = self.pk[:, O_WC:O_WC + NK * D]
        cto = self.pk[:, O_CTO:O_CTO + NK * (D + 1)]
        self.cto_v = cto.rearrange("p (k e) -> p k e", k=NK)
        self.UTt = ipool.tile([D, CL], BF16, tag=f"UTt{b}")
        self.UT = ipool.tile([D, CL], BF16, tag=f"UT{b}")
        self.rmul = ipool.tile([D, NK * D], BF16, tag=f"rmul{b}")
        self.rbias = ipool.tile([D, NK], F32, tag=f"rbias{b}")
        # pooled (bufs=2) per-batch working tiles
        self.E = pool.tile([D, NK * QL], BF16, tag="E")
        self.En = pool.tile([D, NK * QL], BF16, tag="En")
        self.F = pool.tile([D, NJ * CL], BF16, tag="F", bufs=3)
        self.Fr = self.F.rearrange("p (k j c) -> p k j c", k=NK, j=NJ)
        self.Fh = self.F.rearrange("p (h m c) -> p h m c", h=2, m=NK * NJ // 2)
        self.rowsum = pool.tile([D, NK], F32, tag="rowsum")
        self.rsi = pool.tile([D, NK], F32, tag="rsi")
        self.Gn = pool.tile([D, NJ * D], BF16, tag="Gn")
        self.crec = pool.tile([D, NJ], F32, tag="crec")


def _upfront(nc, t):
    # UT = wq^T + wqc^T * C  (GPSIMD, bf16 out) — batch-independent
    nc.gpsimd.tensor_mul(t.UTt[:], t.wqcT[:], t.C)
    nc.gpsimd.tensor_add(t.UT[:], t.UTt[:], t.wqT[:])
    # rbias[c] = sum_d wc * C^T  (DVE, c-part chunks)
    nc.vector.tensor_mul(t.rmul[:], t.wc[:], t.cto_v[:, :, 0:D])
    nc.vector.tensor_reduce(t.rbias[:],
                            t.rmul.rearrange("p (k e) -> p k e", k=NK),
                            axis=mybir.AxisListType.X, op=mybir.AluOpType.add)


def _stage1(nc, t, psS):
    # S chunks -> E = exp(S + rbias), rowsum accum (bf16)
    for k in range(NK):
        ps = psS.tile([D, QL], F32, tag="ps")
        nc.tensor.matmul(ps[:], t.UT[:, k * D:(k + 1) * D], t.Qb[:],
                         start=True, stop=True)
        nc.scalar.activation(t.E[:, k * QL:(k + 1) * QL], ps[:], EXP,
                             bias=t.rbias[:, k:k + 1],
                             accum_out=t.rowsum[:, k:k + 1])
        if k == NK // 2 - 1 or k == NK - 1:
            # normalize this half: En = E * (1/rowsum) per c-partition.
            # En == S1 exactly, so En^T feeds A/B with no output scaling
            # and no rrB broadcast (kills two sync-ring DMAs per batch).
            hh = k // (NK // 2)
            ksl = slice(hh * NK // 2, (hh + 1) * NK // 2)
            nc.vector.reciprocal(t.rsi[:, ksl], t.rowsum[:, ksl])
            for kk in range(hh * NK // 2, (hh + 1) * NK // 2):
                nc.vector.tensor_scalar_mul(
                    t.En[:, kk * QL:(kk + 1) * QL],
                    t.E[:, kk * QL:(kk + 1) * QL], t.rsi[:, kk:kk + 1])


def _stage2x(nc, t):
    # F = En^T (== S1^T) via DMA xbar transpose per half (sync ring)
    for h in range(2):
        nc.sync.dma_start_transpose(
            t.Fh[:, h, :, :], t.En[:, h * 4 * QL:(h + 1) * 4 * QL])


def _stage2g(nc, t, psG):
    # G = E^T @ [C^T|1] per q-chunk; Gn = G/colsum (bf16)
    for j in range(NJ):
        psg = psG.tile([D, D + 1], F32, tag="psg")
        for k in range(NK):
            nc.tensor.matmul(psg[:],
                             t.E[:, k * QL + j * D: k * QL + (j + 1) * D],
                             t.cto_v[:, k, :],
                             start=(k == 0), stop=(k == NK - 1))
        nc.vector.reciprocal(t.crec[:, j:j + 1], psg[:, D:D + 1])
        nc.vector.tensor_scalar_mul(t.Gn[:, j * D:(j + 1) * D], psg[:, 0:D],
                                    t.crec[:, j:j + 1])


def _stage3(nc, t, psAB):
    Asb = t.OB[:, CL:2 * CL]
    CA = t.OB[:, 2 * CL:3 * CL]
    CB = t.OB[:, 3 * CL:4 * CL]
    for h in range(2):
        hs = slice(h * QL, (h + 1) * QL)
        psa = psAB.tile([D, QL], F32, tag="ps")
        for j in range(NJ):
            nc.tensor.matmul(psa[:], t.Qt[:, j * D:(j + 1) * D],
                             t.Fr[:, 4 * h:4 * h + 4, j, :],
                             start=(j == 0), stop=(j == NJ - 1))
        nc.vector.tensor_copy(Asb[:, hs], psa[:])
        nc.vector.tensor_mul(CA[:, hs], psa[:], t.C[:, hs])
        psb = psAB.tile([D, QL], F32, tag="ps")
        for j in range(NJ):
            nc.tensor.matmul(psb[:], t.Gn[:, j * D:(j + 1) * D],
                             t.Fr[:, 4 * h:4 * h + 4, j, :],
                             start=(j == 0), stop=(j == NJ - 1))
        nc.vector.tensor_mul(CB[:, hs], psb[:], t.C[:, hs])
    # (output DMA emitted separately, deferred — see _out)


def _out(nc, t):
    # rows 128:512 (A^T, C*A, C*B); row block 0:128 (C) was written upfront.
    # Emitted two batches late on the sync ring: its deps are long done by
    # then, so it never head-of-line blocks the xbar transposes behind it.
    nc.sync.dma_start(t.OUT_d[D:4 * D].rearrange("(r p) c -> p r c", p=D),
                      t.OB[:, CL:].rearrange("p (r c) -> p r c", r=3))


def _build():
    nc = bacc.Bacc("TRN2", debug=False, num_devices=NCORES)

    C_d = nc.dram_tensor("C", [BPC, D, CL], F32, kind="ExternalInput").ap()
    PK_d = nc.dram_tensor("PK", [BPC, D, PKW], BF16, kind="ExternalInput").ap()
    OUT_d = nc.dram_tensor("OUT", [BPC, 4 * D, CL], F32, kind="ExternalOutput").ap()

    with tile.TileContext(nc) as tc:
        with (
            tc.tile_pool(name="ins", bufs=1) as ipool,
            tc.tile_pool(name="work", bufs=2) as pool,
            tc.tile_pool(name="psS", bufs=3, space="PSUM") as psS,
            tc.tile_pool(name="psG", bufs=2, space="PSUM") as psG,
            tc.tile_pool(name="psAB", bufs=3, space="PSUM") as psAB,
            tc.tile_pool(name="dram", bufs=2, space="DRAM") as dram,
        ):
            ts = [_Batch(nc, ipool, pool, dram, b, C_d, PK_d, OUT_d)
                  for b in range(BPC)]
            for t in ts:
                # C passthrough output can go as soon as C is loaded
                nc.sync.dma_start(t.OUT_d[0:D, :], t.C)
            # HAM warm-up: keep the PE busy through the ~19us load phase so
            # the clock gate is at 8/8 (2.4 GHz) when real matmuls start
            dmyL = ipool.tile([D, D], BF16, tag="dmyL")
            dmyR = ipool.tile([D, QL], BF16, tag="dmyR")
            nc.vector.memset(dmyL[:], 0.0)
            nc.vector.memset(dmyR[:], 0.0)
            for _ in range(36):
                psd = psS.tile([D, QL], F32, tag="ps")
                nc.tensor.matmul(psd[:], dmyL[:], dmyR[:],
                                 start=True, stop=True)
            for t in ts:
                _upfront(nc, t)
            # Pipeline depth 3: when each instruction reaches its engine
            # queue head, its deps are already satisfied — PE never stalls.
            # PE order: S_b, G_{b-1}, AB_{b-2}, S_{b+1}, ...
            for b in range(BPC):
                _stage1(nc, ts[b], psS)
                _stage2x(nc, ts[b])
                if b >= 1:
                    _stage2g(nc, ts[b - 1], psG)
                if b >= 2:
                    _stage3(nc, ts[b - 2], psAB)
                    _out(nc, ts[b - 2])
            _stage2g(nc, ts[BPC - 1], psG)
            _stage3(nc, ts[BPC - 2], psAB)
            _out(nc, ts[BPC - 2])
            _stage3(nc, ts[BPC - 1], psAB)
            _out(nc, ts[BPC - 1])
    nc.compile()
    return nc


def _get_nc():
    global _NC
    if _NC is None:
        _NC = _build()
    return _NC


def _prep_core(Ci, Qi, Wi):
    """Host-side layout prep for one core's shard (pure transposes/casts)."""
    bpc = Ci.shape[0]
    pk = np.empty((bpc, D, PKW), dtype=BF)
    pk[:, :, O_QB:O_QB + QL] = Qi.astype(BF)
    qt = Qi.transpose(0, 2, 1).reshape(bpc, NJ, D, D).transpose(0, 2, 1, 3)
    pk[:, :, O_QT:O_QT + NJ * D] = qt.reshape(bpc, D, NJ * D).astype(BF)
    pk[:, :, O_WQT:O_WQT + CL] = Wi[:, :, 0:D].transpose(0, 2, 1).astype(BF)
    pk[:, :, O_WQCT:O_WQCT + CL] = (
        Wi[:, :, 2 * D:3 * D].transpose(0, 2, 1).astype(BF))
    pk[:, :, O_WC:O_WC + NK * D] = (
        Wi[:, :, D:2 * D].reshape(bpc, NK, D, D).transpose(0, 2, 1, 3)
        .reshape(bpc, D, NK * D).astype(BF))
    ct = Ci.transpose(0, 2, 1).reshape(bpc, NK, D, D).transpose(0, 2, 1, 3)
    cto = np.concatenate(
        [ct, np.ones((bpc, D, NK, 1), dtype=np.float32)], axis=3)
    pk[:, :, O_CTO:O_CTO + NK * (D + 1)] = (
        cto.reshape(bpc, D, NK * (D + 1)).astype(BF))
    return {"C": np.ascontiguousarray(Ci), "PK": pk}


def kernel(C, Q, W):
    C = np.ascontiguousarray(np.asarray(C, dtype=np.float32))
    Q = np.ascontiguousarray(np.asarray(Q, dtype=np.float32))
    W = np.ascontiguousarray(np.asarray(W, dtype=np.float32)).reshape(B, CL, 3 * D)
    in_maps = [
        _prep_core(C[i * BPC:(i + 1) * BPC],
                   Q[i * BPC:(i + 1) * BPC],
                   W[i * BPC:(i + 1) * BPC])
        for i in range(NCORES)
    ]
    nc = _get_nc()
    res = run_bass_kernel_spmd(nc, in_maps, core_ids=list(range(NCORES)), **RUN_KWARGS)
    global LAST_RESULT
    LAST_RESULT = res
    out = np.concatenate([res.results[i]["OUT"] for i in range(NCORES)], axis=0)
    return out
